# revision 9
# baseline (speedup 1.0000x reference)
"""GumbelVectorQuantizer forward on 8 Trainium2 NeuronCores (Bass/Tile).

Problem: x[32,2000,1024] -> logits = x@W.T (G*V=640), per (token, group):
  - hard one-hot (argmax of logits) -> code perplexity histogram
  - softmax(logits) accumulated      -> avg prob histogram
  - argmax(logits + gumbel)          -> codebook row gather -> q
Outputs (q [32,2000,256], code_perplexity, prob_perplexity).

Sharding: data-parallel over the 64000 tokens, 8000/core; projection
weights and codebook replicated. Tiny [G,V] histograms reduced on host.

Matmul runs in float32r (TF32-like, 1 cyc/row vs 4 for fp32). f32r logit
error (~3e-2 max) can flip argmaxes on near-tie rows, so the device also
emits per-token top-2 values (both argmax paths) + the hard argmax index;
the host exactly recomputes tokens whose top-2 gap < TAU_FLAG and patches
q rows / hard counts. Softmax-histogram errors average out (~1e-4).
"""

import sys

sys.path.insert(0, "/opt/pypackages")
sys.path.insert(0, "/opt/trn_rl_repo")

import numpy as np
from contextlib import ExitStack

import concourse.bass as bass
import concourse.tile as tile
import concourse.mybir as mybir
from concourse.bass_utils import run_bass_kernel_spmd

F32 = mybir.dt.float32
F32R = mybir.dt.float32r
BF16 = mybir.dt.bfloat16
U32 = mybir.dt.uint32
AF = mybir.ActivationFunctionType

# problem constants (hardcoded per contract)
B, T, F_IN = 32, 2000, 1024
GROUPS, NUM_VARS, VAR_DIM = 2, 320, 128
GV = GROUPS * NUM_VARS  # 640
BT = B * T  # 64000
N_CORES = 8
NT = BT // N_CORES  # 8000 tokens per core
EPS = 1e-7
TAU_FLAG = 0.15  # host-refinement threshold on top-2 gap

_CACHE = {}


def _split_excess_waits(nc) -> int:
    """Walrus accepts a single sync wait per instruction (2 on Activation).
    Tile sometimes emits more; hoist the excess onto same-engine NoOps
    inserted just before (engine blocks on the same sems at the same point,
    so scheduling semantics are unchanged)."""
    n = 0
    for f in nc.m.functions:
        for blk in f.blocks:
            il = blk.instructions
            i = 0
            while i < len(il):
                inst = il[i]
                si = inst.sync_info
                if si is not None and len(si.on_wait) > 1:
                    waits = list(si.on_wait)
                    inst.sync_info = mybir.SyncInfo(
                        on_wait=waits[-1:], on_update=list(si.on_update)
                    )
                    pos = i
                    for w in waits[:-1]:
                        nop = mybir.InstNoOp(name=f"I-wsplit-{n}", ins=[], outs=[])
                        n += 1
                        nop.engine = inst.engine
                        nop.sync_info = mybir.SyncInfo(on_wait=[w], on_update=[])
                        il.insert(pos, nop)
                        pos += 1
                        i += 1
                i += 1
    return n


def build_nc(mm_dtype=F32R, with_bias=False, nt=NT):
    nc = bass.Bass("TRN2", target_bir_lowering=False, debug=False, num_devices=N_CORES)

    xT = nc.dram_tensor("xT", [8, 128, nt], mm_dtype, kind="ExternalInput").ap()
    gm = nc.dram_tensor("gm", [nt, GV], F32, kind="ExternalInput").ap()
    wT = nc.dram_tensor("wT", [128, 8 * GV], mm_dtype, kind="ExternalInput").ap()
    cbs = [
        nc.dram_tensor(f"cb{g}", [NUM_VARS, VAR_DIM], F32, kind="ExternalInput").ap()
        for g in range(GROUPS)
    ]
    if with_bias:
        bvec = nc.dram_tensor("bvec", [1, GV], mm_dtype, kind="ExternalInput").ap()

    q_out = nc.dram_tensor("q_out", [nt, 2 * VAR_DIM], F32, kind="ExternalOutput").ap()
    st_out = nc.dram_tensor("st_out", [1, 4 * NUM_VARS], F32, kind="ExternalOutput").ap()
    ms_out = nc.dram_tensor("ms_out", [nt, 32], F32, kind="ExternalOutput").ap()
    hx_out = nc.dram_tensor("hx_out", [nt, 16], U32, kind="ExternalOutput").ap()

    ntiles = (nt + 127) // 128

    with tile.TileContext(nc) as tc, ExitStack() as ctx:
        consts = ctx.enter_context(tc.tile_pool(name="consts", bufs=1))
        xp = ctx.enter_context(tc.tile_pool(name="xp", bufs=4))
        gp = ctx.enter_context(tc.tile_pool(name="gp", bufs=4))
        sp = ctx.enter_context(tc.tile_pool(name="sp", bufs=4))
        qp = ctx.enter_context(tc.tile_pool(name="qp", bufs=4))
        pp = ctx.enter_context(tc.tile_pool(name="pp", bufs=2, space="PSUM"))
        ap_ = ctx.enter_context(tc.tile_pool(name="accp", bufs=1, space="PSUM"))

        wt_sb = consts.tile([128, 8 * GV], mm_dtype)
        for c in range(8):
            nc.sync.dma_start(
                wt_sb[:, c * GV : (c + 1) * GV], wT[:, c * GV : (c + 1) * GV]
            )
        ones_bf = consts.tile([128, 1], BF16)
        nc.vector.memset(ones_bf[:], 1.0)
        if with_bias:
            b_sb = consts.tile([1, GV], mm_dtype)
            nc.sync.dma_start(b_sb[:], bvec[:])
            ones_f = consts.tile([1, 128], mm_dtype)
            nc.vector.memset(ones_f[:], 1.0)

        # four independent single-partition accumulation regions (PE output
        # base partition must be 0/32/64; one PSUM bank each)
        acc_p = [ap_.tile([1, NUM_VARS], F32, name=f"accp{g}", tag=f"accp{g}") for g in range(2)]
        acc_h = [ap_.tile([1, NUM_VARS], F32, name=f"acch{g}", tag=f"acch{g}") for g in range(2)]

        for i in range(ntiles):
            t0 = i * 128
            P = min(128, nt - t0)

            xt = xp.tile([128, 1024], mm_dtype)
            xt_v = xt[:].rearrange("p (c j) -> p c j", c=8)[:, :, 0:P]
            nc.sync.dma_start(xt_v, xT[:, :, t0 : t0 + P].rearrange("c p t -> p c t"))
            gmt = gp.tile([128, GV], F32)
            nc.sync.dma_start(gmt[0:P, :], gm[t0 : t0 + P, :])

            lg_ps = []
            for g in range(2):
                ps = pp.tile([128, NUM_VARS], F32, tag=f"lg{g}")
                for c in range(8):
                    nc.tensor.matmul(
                        ps[0:P, :],
                        lhsT=xt[:, c * 128 : c * 128 + P],
                        rhs=wt_sb[:, c * GV + g * NUM_VARS : c * GV + (g + 1) * NUM_VARS],
                        start=(c == 0),
                        stop=(c == 7) if not with_bias else False,
                    )
                if with_bias:
                    nc.tensor.matmul(
                        ps[0:P, :],
                        lhsT=ones_f[:, 0:P],
                        rhs=b_sb[:, g * NUM_VARS : (g + 1) * NUM_VARS],
                        start=False,
                        stop=True,
                    )
                lg_ps.append(ps)

            lg_sb = sp.tile([128, GV], F32, tag="lg_sb")
            ms = sp.tile([128, 32], F32, tag="ms")
            nm = sp.tile([128, 2], F32, tag="nm")
            ex = sp.tile([128, GV], BF16, tag="ex")
            ss = sp.tile([128, 2], F32, tag="ss")
            rb = sp.tile([128, 2], BF16, tag="rb")
            hx = sp.tile([128, 16], U32, tag="hx")
            yx = sp.tile([128, 16], U32, tag="yx")
            lgg = sp.tile([128, GV], F32, tag="lgg")
            mk = sp.tile([128, GV], BF16, tag="mk")
            qt = qp.tile([128, 2 * VAR_DIM], F32, tag="qt")

            for g in range(2):
                gs = slice(g * NUM_VARS, (g + 1) * NUM_VARS)
                m8 = ms[:, g * 8 : (g + 1) * 8]
                # PSUM readers: DVE max8, ACT copy, ACT exp
                nc.vector.max(m8[0:P, :], lg_ps[g][0:P, :])
                nc.scalar.copy(lg_sb[0:P, gs], lg_ps[g][0:P, :])
                nc.scalar.mul(nm[0:P, g : g + 1], ms[0:P, g * 8 : g * 8 + 1], -1.0)
                nc.scalar.activation(
                    ex[0:P, gs],
                    lg_ps[g][0:P, :],
                    AF.Exp,
                    bias=nm[0:P, g : g + 1],
                    scale=1.0,
                    accum_out=ss[0:P, g : g + 1],
                )
                nc.vector.max_index(hx[0:P, g * 8 : (g + 1) * 8], m8[0:P, :], lg_sb[0:P, gs])
                # hard one-hot as mask (ties are measure-zero)
                nc.gpsimd.tensor_scalar(
                    mk[0:P, gs],
                    lg_sb[0:P, gs],
                    ms[0:P, g * 8 : g * 8 + 1],
                    None,
                    op0=mybir.AluOpType.is_equal,
                )

            # probs / hard-count accumulators: PE matmuls into persistent PSUM
            for g in range(2):
                gs = slice(g * NUM_VARS, (g + 1) * NUM_VARS)
                with nc.allow_low_precision(
                    "bf16 1/s feeds a bf16 prob-histogram matmul; stats only"
                ):
                    nc.vector.reciprocal(rb[0:P, g : g + 1], ss[0:P, g : g + 1])
                nc.tensor.matmul(
                    acc_p[g][:],
                    lhsT=rb[0:P, g : g + 1],
                    rhs=ex[0:P, gs],
                    start=(i == 0),
                    stop=(i == ntiles - 1),
                    skip_group_check=True,
                )
                nc.tensor.matmul(
                    acc_h[g][:],
                    lhsT=ones_bf[0:P, :],
                    rhs=mk[0:P, gs],
                    start=(i == 0),
                    stop=(i == ntiles - 1),
                    skip_group_check=True,
                )

            # gumbel path
            nc.gpsimd.tensor_tensor(
                lgg[0:P, :], lg_sb[0:P, :], gmt[0:P, :], op=mybir.AluOpType.add
            )
            for g in range(2):
                gs = slice(g * NUM_VARS, (g + 1) * NUM_VARS)
                my8 = ms[:, 16 + g * 8 : 24 + g * 8]
                nc.vector.max(my8[0:P, :], lgg[0:P, gs])
                nc.vector.max_index(yx[0:P, g * 8 : (g + 1) * 8], my8[0:P, :], lgg[0:P, gs])
                nc.gpsimd.indirect_dma_start(
                    out=qt[0:P, g * VAR_DIM : (g + 1) * VAR_DIM],
                    out_offset=None,
                    in_=cbs[g][:],
                    in_offset=bass.IndirectOffsetOnAxis(ap=yx[0:P, g * 8 : g * 8 + 1], axis=0),
                )

            nc.sync.dma_start(q_out[t0 : t0 + P, :], qt[0:P, :])
            nc.sync.dma_start(ms_out[t0 : t0 + P, :], ms[0:P, :])
            nc.sync.dma_start(hx_out[t0 : t0 + P, :], hx[0:P, :])

        # finalize stats: one partition-0 row [1, 4*320]
        st_sb = consts.tile([1, 4 * NUM_VARS], F32)
        for g in range(2):
            nc.scalar.copy(st_sb[:, g * NUM_VARS : (g + 1) * NUM_VARS], acc_p[g][:])
            nc.scalar.copy(
                st_sb[:, (2 + g) * NUM_VARS : (3 + g) * NUM_VARS], acc_h[g][:]
            )
        nc.sync.dma_start(st_out[:], st_sb[:])

    _split_excess_waits(nc)
    return nc


last_results = None  # BassKernelResults of the most recent device run (for test.py)
last_in_maps = None  # per-core input dicts of the most recent kernel() call


def bench_device(n_warm=2, n_iters=10):
    """Wall-clock the device executable with pre-staged inputs.

    NTFF profiling isn't available through this container's axon build, so
    this is the timing source: build the shard_map'd executable once,
    device_put inputs once, then time n_iters back-to-back executions
    (async dispatch, single final block) and per-call blocking executions.
    Returns (batch_ns_per_iter, percall_ns_min).
    """
    import time
    import jax
    import jax.numpy as jnp
    from jax.sharding import Mesh, PartitionSpec, NamedSharding
    from jax.experimental.shard_map import shard_map
    from concourse import bass2jax
    from concourse.bass2jax import _bass_exec_p, partition_id_tensor
    import concourse.mybir as mybir_

    assert last_in_maps is not None, "run kernel() first"
    nc = next(iter(_CACHE.values()))
    in_maps = last_in_maps
    n_cores = N_CORES

    partition_name = nc.partition_id_tensor.name if nc.partition_id_tensor else None
    in_names, out_names, out_avals, zero_outs = [], [], [], []
    for alloc in nc.m.functions[0].allocations:
        if not isinstance(alloc, mybir_.MemoryLocationSet):
            continue
        name = alloc.memorylocations[0].name
        if alloc.kind == "ExternalInput":
            if name != partition_name:
                in_names.append(name)
        elif alloc.kind == "ExternalOutput":
            shape = tuple(alloc.tensor_shape)
            dtype = mybir_.dt.np(alloc.dtype)
            out_names.append(name)
            out_avals.append(jax.core.ShapedArray(shape, dtype))
            zero_outs.append(np.zeros(shape, dtype))
    n_params = len(in_names)
    all_in_names = list(in_names) + list(out_names)
    if partition_name is not None:
        all_in_names.append(partition_name)

    def _body(*args):
        operands = list(args)
        if partition_name is not None:
            operands.append(partition_id_tensor())
        outs = _bass_exec_p.bind(
            *operands,
            out_avals=tuple(out_avals),
            in_names=tuple(all_in_names),
            out_names=tuple(out_names),
            lowering_input_output_aliases=(),
            sim_require_finite=True,
            sim_require_nnan=True,
            nc=nc,
        )
        return tuple(outs)

    devices = jax.devices()[:n_cores]
    mesh = Mesh(np.asarray(devices), ("core",))
    in_specs = (PartitionSpec("core"),) * (n_params + len(out_names))
    out_specs = (PartitionSpec("core"),) * len(out_names)
    fn = jax.jit(
        shard_map(_body, mesh=mesh, in_specs=in_specs, out_specs=out_specs,
                  check_rep=False),
        keep_unused=True,
    )
    sh = NamedSharding(mesh, PartitionSpec("core"))
    concat_in = [
        jax.device_put(
            np.concatenate([np.asarray(in_maps[c][n]) for c in range(n_cores)], 0), sh
        )
        for n in in_names
    ]
    concat_zeros = [
        jax.device_put(np.zeros((n_cores * z.shape[0], *z.shape[1:]), z.dtype), sh)
        for z in zero_outs
    ]
    for _ in range(n_warm):
        out = fn(*concat_in, *concat_zeros)
        jax.block_until_ready(out)
    # batch: async dispatch, single final block
    t0 = time.perf_counter()
    outs = [fn(*concat_in, *concat_zeros) for _ in range(n_iters)]
    jax.block_until_ready(outs)
    t1 = time.perf_counter()
    batch_ns = (t1 - t0) / n_iters * 1e9
    # per-call minimum
    best = float("inf")
    for _ in range(n_iters):
        t0 = time.perf_counter()
        out = fn(*concat_in, *concat_zeros)
        jax.block_until_ready(out)
        best = min(best, time.perf_counter() - t0)
    return batch_ns, best * 1e9


def _get_nc(with_bias):
    key = ("f32r", with_bias)
    if key not in _CACHE:
        _CACHE[key] = build_nc(F32R, with_bias)
    return _CACHE[key]


def kernel(x, proj_w, proj_b, codebook, gumbel):
    global last_results
    x = np.ascontiguousarray(x, dtype=np.float32)
    proj_w = np.ascontiguousarray(proj_w, dtype=np.float32)
    proj_b = np.ascontiguousarray(proj_b, dtype=np.float32)
    codebook = np.ascontiguousarray(codebook, dtype=np.float32)
    gumbel = np.ascontiguousarray(gumbel, dtype=np.float32)

    with_bias = bool(np.any(proj_b))
    nc = _get_nc(with_bias)

    X2 = x.reshape(BT, F_IN)
    XT = np.ascontiguousarray(X2.T)  # [1024, BT]
    wTa = np.ascontiguousarray(
        proj_w.T.reshape(8, 128, GV).transpose(1, 0, 2)
    ).reshape(128, 8 * GV)
    cb = codebook.reshape(GROUPS, NUM_VARS, VAR_DIM)

    in_maps = []
    for c in range(N_CORES):
        sl = slice(c * NT, (c + 1) * NT)
        m = {
            "xT": np.ascontiguousarray(XT[:, sl]).reshape(8, 128, NT),
            "gm": gumbel[sl],
            "wT": wTa,
            "cb0": np.ascontiguousarray(cb[0]),
            "cb1": np.ascontiguousarray(cb[1]),
        }
        if with_bias:
            m["bvec"] = proj_b.reshape(1, GV)
        in_maps.append(m)

    global last_in_maps
    last_in_maps = in_maps
    last_results = run_bass_kernel_spmd(nc, in_maps, list(range(N_CORES)))
    res = last_results.results

    q = np.concatenate([r["q_out"] for r in res], axis=0)  # [BT, 256]
    acc_p = np.zeros((2, NUM_VARS), np.float64)
    counts = np.zeros((2, NUM_VARS), np.float64)
    for r in res:
        st = r["st_out"].reshape(4, NUM_VARS)
        acc_p += st[0:2]
        counts += st[2:4]
    ms = np.concatenate([r["ms_out"] for r in res], axis=0)  # [BT, 32]
    hx = np.concatenate([r["hx_out"] for r in res], axis=0)  # [BT, 16]

    # --- host refinement of near-tie tokens (f32r rounding) ---
    gaps = np.stack(
        [
            ms[:, 0] - ms[:, 1],
            ms[:, 8] - ms[:, 9],
            ms[:, 16] - ms[:, 17],
            ms[:, 24] - ms[:, 25],
        ],
        axis=1,
    )
    idxs = np.nonzero(gaps.min(axis=1) < TAU_FLAG)[0]
    if idxs.size:
        lgx = (
            X2[idxs].astype(np.float64) @ proj_w.astype(np.float64).T
            + proj_b.astype(np.float64)
        )
        lggx = lgx + gumbel[idxs].astype(np.float64)
        for g in range(2):
            sl = slice(g * NUM_VARS, (g + 1) * NUM_VARS)
            kh = np.argmax(lgx[:, sl], axis=1)
            dev_kh = hx[idxs, g * 8].astype(np.int64)
            np.add.at(counts[g], kh, 1.0)
            np.add.at(counts[g], np.clip(dev_kh, 0, NUM_VARS - 1), -1.0)
            ky = np.argmax(lggx[:, sl], axis=1)
            q[idxs, g * VAR_DIM : (g + 1) * VAR_DIM] = cb[g][ky]

    hard_probs = (counts / BT).astype(np.float32)
    avg_probs = (acc_p / BT).astype(np.float32)
    code_ppl = np.exp(
        -np.sum(hard_probs * np.log(hard_probs + EPS), axis=-1)
    ).sum()
    prob_ppl = np.exp(-np.sum(avg_probs * np.log(avg_probs + EPS), axis=-1)).sum()

    q_full = np.ascontiguousarray(q.reshape(B, T, 2 * VAR_DIM), dtype=np.float32)
    return q_full, np.float32(code_ppl), np.float32(prob_ppl)
